# revision 12
# baseline (speedup 1.0000x reference)
"""Trainium2 Bass kernel for a single-layer transformer encoder.

Model: B=2, N=2048, D=1024, H=16, DFF=4096 (pre-computed QKV attention +
residual/LN + GELU FFN + residual/LN).

Sharding (zero-collective): 2 batches x 4-way sequence split. Core c owns
the 512 query tokens q=c%4 of batch b=c//4 and recomputes K/V for its whole
batch locally (~1.37x compute redundancy, but no collectives at all).

Device layout is feature-major ("transposed"): activations are stored as
[feature, token] so every projection's weight matrix is the natural
stationary (lhsT) operand and activations stream as the moving operand at
the fp32r full-rate free-dim of 512. Softmax runs on transposed scores
PT[j, i] = exp(scale * k_j . q_i); the denominators come for free from a
ones-column appended to V (out partition 64 of the attention-output
accumulation), so no cross-partition reduction is ever needed. LayerNorm
reductions over the feature (partition) dim are done with ones-vector
matmuls on the PE; per-token mean/rstd are broadcast back across
partitions with rank-1 (k=1) matmuls.

All matmuls run in float32r (TF32-like, full PE rate at free-dim >= 256,
~1.5e-4 relative error per 1024-deep contraction).
"""

import os
import sys

for _p in ("/opt/trn_rl_repo", "/root/.axon_site", "/root/.axon_site/_ro/trn_rl_repo"):
    if os.path.isdir(_p) and _p not in sys.path:
        sys.path.append(_p)

import numpy as np

import concourse.bacc as bacc
import concourse.mybir as mybir
import concourse.tile as tile
from concourse.bass_utils import run_bass_kernel_spmd

P = 128
B, NSEQ, D, H, DFF = 2, 2048, 1024, 16, 4096
DH = D // H                     # 64
NT = 512                        # query tokens per core
DM = D // P                     # 8 feature chunks
JC = NSEQ // P                  # 16 key-token chunks
TC = NSEQ // 512                # 4 512-token chunks
FC = DFF // P                   # 32 FFN feature chunks
HPAIRS = H // 2                 # 8
SCALE = DH ** -0.5
EPS = 1e-5

F32 = mybir.dt.float32
F32R = mybir.dt.float32r
AF = mybir.ActivationFunctionType

_NC_CACHE = None


def _rearr(ap):
    """DRAM [D_like, T] -> [p, chunk, T] view with chunk-major features."""
    return ap.rearrange("(c p) t -> p c t", p=P)


def _build_nc():
    nc = bacc.Bacc("TRN2", target_bir_lowering=False, debug=False)

    xT = nc.dram_tensor("xT", [D, NSEQ], F32R, kind="ExternalInput")
    w_qkv = nc.dram_tensor("w_qkv", [D, 3 * D], F32R, kind="ExternalInput")
    w_out = nc.dram_tensor("w_out", [D, D], F32R, kind="ExternalInput")
    w1 = nc.dram_tensor("w1", [D, DFF], F32R, kind="ExternalInput")
    w2 = nc.dram_tensor("w2", [DFF, D], F32R, kind="ExternalInput")
    b1 = nc.dram_tensor("b1", [DFF], F32, kind="ExternalInput")
    b2 = nc.dram_tensor("b2", [D], F32, kind="ExternalInput")
    ln1_w = nc.dram_tensor("ln1_w", [D], F32, kind="ExternalInput")
    ln1_b = nc.dram_tensor("ln1_b", [D], F32, kind="ExternalInput")
    ln2_w = nc.dram_tensor("ln2_w", [D], F32, kind="ExternalInput")
    ln2_b = nc.dram_tensor("ln2_b", [D], F32, kind="ExternalInput")
    yT = nc.dram_tensor("yT", [D, NT], F32, kind="ExternalOutput")

    with tile.TileContext(nc) as tc, \
         nc.allow_low_precision(reason="float32r tensors feed fp32r matmuls"):
        _emit(nc, tc, xT, w_qkv, w_out, w1, w2, b1, b2,
              ln1_w, ln1_b, ln2_w, ln2_b, yT)
    nc.compile()
    return nc


def _emit(nc, tc, xT_d, w_qkv, w_out, w1, w2, b1, b2,
          ln1_w, ln1_b, ln2_w, ln2_b, yT_d):
    # ---------------- whole-kernel pools ----------------
    with tc.tile_pool(name="const", bufs=1) as pc, \
         tc.tile_pool(name="pers", bufs=1) as pers, \
         tc.tile_pool(name="scratch", bufs=2) as sq_pool, \
         tc.tile_pool(name="vecs", bufs=4) as vec_pool, \
         tc.tile_pool(name="psacc", bufs=2, space="PSUM") as psacc, \
         tc.tile_pool(name="pspt", bufs=2, space="PSUM") as pspt, \
         tc.tile_pool(name="psout", bufs=3, space="PSUM") as psout, \
         tc.tile_pool(name="psb", bufs=1, space="PSUM") as psb:

        # ---------------- constants ----------------
        ones_f32 = pc.tile([P, P], F32)
        nc.vector.memset(ones_f32[:], 1.0)
        ones_col = pc.tile([P, 1], F32R)          # lhsT for partition-sums
        nc.vector.tensor_copy(ones_col[:], ones_f32[:, 0:1])
        ones_row = pc.tile([1, P], F32R)          # lhsT for partition-broadcasts
        nc.vector.tensor_copy(ones_row[:], ones_f32[0:1, :])
        eps_sb = pc.tile([1, 1], F32)
        nc.vector.memset(eps_sb[:], EPS)
        b1_sb = pc.tile([P, FC], F32)
        nc.sync.dma_start(b1_sb[:], b1.ap().rearrange("(c p) -> p c", p=P))
        b2_sb = pc.tile([P, DM], F32)
        nc.sync.dma_start(b2_sb[:], b2.ap().rearrange("(c p) -> p c", p=P))
        lnw1_sb = pc.tile([P, DM], F32)
        nc.sync.dma_start(lnw1_sb[:], ln1_w.ap().rearrange("(c p) -> p c", p=P))
        lnb1_sb = pc.tile([P, DM], F32)
        nc.sync.dma_start(lnb1_sb[:], ln1_b.ap().rearrange("(c p) -> p c", p=P))
        lnw2_sb = pc.tile([P, DM], F32)
        nc.sync.dma_start(lnw2_sb[:], ln2_w.ap().rearrange("(c p) -> p c", p=P))
        lnb2_sb = pc.tile([P, DM], F32)
        nc.sync.dma_start(lnb2_sb[:], ln2_b.ap().rearrange("(c p) -> p c", p=P))

        # Three persistent [P, DM, NT] buffers, each reused across phases:
        #   QT   (q-projection)        -> z1   (attn residual sum)
        #   outT (attention output)    -> xln1 (LN1 output)
        #   xow  (own-token x slice)   -> z2   (ffn residual sum)
        QT = pers.tile([P, DM, NT], F32R)
        outT = pers.tile([P, DM, NT], F32R)
        xow = pers.tile([P, DM, NT], F32R)
        z1, xln1, z2 = QT, outT, xow

        def ln_apply(z_tile, dst_write):
            """LayerNorm over the feature (partition x chunk) axis of
            z_tile [P, DM, NT].  dst_write(k, src_ap) stores chunk k."""
            s1 = psacc.tile([1, NT], F32, tag="acc")
            for k in range(DM):
                nc.tensor.matmul(s1[:], ones_col[:], z_tile[:, k, :],
                                 start=(k == 0), stop=(k == DM - 1))
            s2 = psacc.tile([1, NT], F32, tag="acc")
            for k in range(DM):
                sq = sq_pool.tile([P, NT], F32R, tag="sq")
                nc.scalar.activation(sq[:], z_tile[:, k, :], AF.Square)
                nc.tensor.matmul(s2[:], ones_col[:], sq[:],
                                 start=(k == 0), stop=(k == DM - 1))
            mu = vec_pool.tile([1, NT], F32, tag="v")
            nc.vector.tensor_scalar_mul(mu[:], s1[:], 1.0 / D)
            var = vec_pool.tile([1, NT], F32, tag="v")
            nc.vector.tensor_scalar_mul(var[:], s2[:], 1.0 / D)
            musq = vec_pool.tile([1, NT], F32, tag="v")
            nc.vector.tensor_mul(musq[:], mu[:], mu[:])
            nc.vector.tensor_sub(var[:], var[:], musq[:])
            nc.scalar.activation(var[:], var[:], AF.Sqrt, bias=eps_sb[:])
            r = vec_pool.tile([1, NT], F32R, tag="v")
            nc.vector.reciprocal(r[:], var[:])
            mur = vec_pool.tile([1, NT], F32R, tag="v")
            nc.vector.tensor_mul(mur[:], mu[:], r[:])
            R = psacc.tile([P, NT], F32, tag="acc")
            nc.tensor.matmul(R[:], ones_row[:], r[:], start=True, stop=True)
            MR = psacc.tile([P, NT], F32, tag="acc")
            nc.tensor.matmul(MR[:], ones_row[:], mur[:], start=True, stop=True)
            for k in range(DM):
                t = sq_pool.tile([P, NT], F32, tag="sq")
                nc.vector.tensor_mul(t[:], z_tile[:, k, :], R[:])
                nc.vector.tensor_sub(t[:], t[:], MR[:])
                dst_write(k, t)

        with tc.tile_pool(name="xpool", bufs=1) as px:
            xT = px.tile([P, DM, NSEQ], F32R)
            nc.sync.dma_start(xT[:], _rearr(xT_d.ap()))

            # -------- Q projection (own 512 tokens = xT cols 0:NT) -------
            with tc.tile_pool(name="wq", bufs=2) as wq_pool:
                for qf in range(DM):
                    wq = wq_pool.tile([P, DM, P], F32R)
                    nc.sync.dma_start(
                        wq[:], _rearr(w_qkv.ap()[:, qf * P:(qf + 1) * P]))
                    acc = psacc.tile([P, NT], F32, tag="acc")
                    for k in range(DM):
                        nc.tensor.matmul(acc[:], wq[:, k, :], xT[:, k, 0:NT],
                                         start=(k == 0), stop=(k == DM - 1))
                    nc.vector.tensor_copy(QT[:, qf, :], acc[:])
                # stash own-token x for the attention residual
                for k in range(DM):
                    nc.vector.tensor_copy(xow[:, k, :], xT[:, k, 0:NT])

            # -------- attention: 8 head-pairs, V produced in 2 halves ----
            with tc.tile_pool(name="wk", bufs=2) as wk_pool, \
                 tc.tile_pool(name="wv", bufs=1) as wv_pool, \
                 tc.tile_pool(name="kt", bufs=2) as kt_pool, \
                 tc.tile_pool(name="vp", bufs=1) as vp_pool, \
                 tc.tile_pool(name="pt", bufs=4) as pt_pool:
                vp = None
                vp_h = None
                for hp in range(HPAIRS):
                    if hp % (HPAIRS // 2) == 0:
                        # V projection for heads [8*half, 8*half+8)
                        half = hp // (HPAIRS // 2)
                        wv = wv_pool.tile([P, DM, 512], F32R)
                        nc.sync.dma_start(
                            wv[:],
                            _rearr(w_qkv.ap()[:, 2 * D + half * 512:
                                              2 * D + (half + 1) * 512]))
                        vp = vp_pool.tile([P, JC, 8 * 65], F32R)
                        vp_h = vp.rearrange("p j (h e) -> p j h e", e=65)
                        nc.vector.tensor_copy(
                            vp_h[:, :, :, 64:65],
                            ones_f32.rearrange("p (a b c) -> p a b c",
                                               b=8, c=1))
                        for jc in range(JC):
                            acc = psacc.tile([P, 512], F32, tag="acc")
                            for k in range(DM):
                                nc.tensor.matmul(
                                    acc[:], xT[:, k, jc * P:(jc + 1) * P],
                                    wv[:, k, :],
                                    start=(k == 0), stop=(k == DM - 1))
                            nc.vector.tensor_copy(
                                vp_h[:, jc, :, 0:64],
                                acc[:].rearrange("p (h e) -> p h e", e=64))

                    # K^T projection for this head pair (128 features)
                    kt = kt_pool.tile([P, NSEQ], F32R)
                    wk = wk_pool.tile([P, DM, P], F32R)
                    nc.sync.dma_start(
                        wk[:],
                        _rearr(w_qkv.ap()[:, D + hp * P:D + (hp + 1) * P]))
                    for t in range(TC):
                        acc = psacc.tile([P, 512], F32, tag="acc")
                        for k in range(DM):
                            nc.tensor.matmul(
                                acc[:], wk[:, k, :],
                                xT[:, k, t * 512:(t + 1) * 512],
                                start=(k == 0), stop=(k == DM - 1))
                        nc.vector.tensor_copy(kt[:, t * 512:(t + 1) * 512],
                                              acc[:])

                    # streaming softmax + attn@V for the two heads
                    oacc = [psout.tile([65, NT], F32, tag="o", name=f"oacc{i}")
                            for i in range(2)]
                    hloc = [(2 * hp + i) % 8 for i in range(2)]
                    for jc in range(JC):
                        pts = []
                        for i in range(2):
                            rows = slice(64 * i, 64 * i + 64)
                            pt_ps = pspt.tile([P, NT], F32, tag="pt",
                                              name=f"pt{i}")
                            nc.tensor.matmul(pt_ps[:],
                                             kt[rows, jc * P:(jc + 1) * P],
                                             QT[rows, hp, :],
                                             start=True, stop=True)
                            pts.append(pt_ps)
                        for i in range(2):
                            pt_sb = pt_pool.tile([P, NT], F32R, tag="ptsb",
                                                 name=f"ptsb{i}")
                            nc.scalar.activation(pt_sb[:], pts[i][:], AF.Exp,
                                                 scale=SCALE)
                            nc.tensor.matmul(
                                oacc[i][:],
                                vp[:, jc, hloc[i] * 65:(hloc[i] + 1) * 65],
                                pt_sb[:],
                                start=(jc == 0), stop=(jc == JC - 1))
                    for i in range(2):
                        inv_s = vec_pool.tile([1, NT], F32R, tag="v")
                        nc.vector.reciprocal(inv_s[:], oacc[i][64:65, :])
                        bc = psb.tile([64, NT], F32, tag="b")
                        nc.tensor.matmul(bc[:], ones_row[:, 0:64], inv_s[:],
                                         start=True, stop=True)
                        bc_sb = sq_pool.tile([P, NT], F32, tag="sq")
                        nc.scalar.activation(bc_sb[0:64, :], bc[:], AF.Copy)
                        nc.vector.tensor_mul(outT[64 * i:64 * i + 64, hp, :],
                                             oacc[i][0:64, :], bc_sb[0:64, :])

            # -------- output projection + residual 1 ---------------------
            with tc.tile_pool(name="wo", bufs=2) as wo_pool:
                for ef in range(DM):
                    wo = wo_pool.tile([P, DM, P], F32R)
                    nc.sync.dma_start(
                        wo[:], _rearr(w_out.ap()[:, ef * P:(ef + 1) * P]))
                    acc = psacc.tile([P, NT], F32, tag="acc")
                    for k in range(DM):
                        nc.tensor.matmul(acc[:], wo[:, k, :], outT[:, k, :],
                                         start=(k == 0), stop=(k == DM - 1))
                    nc.vector.tensor_add(z1[:, ef, :], acc[:], xow[:, ef, :])

        # -------- LN1 -> xln1 (overwrites outT) -------------------------
        def write_xln1(k, t):
            nc.scalar.activation(xln1[:, k, :], t[:], AF.Identity,
                                 scale=lnw1_sb[:, k:k + 1],
                                 bias=lnb1_sb[:, k:k + 1])
        ln_apply(z1, write_xln1)

        # -------- FFN ---------------------------------------------------
        with tc.tile_pool(name="hpool", bufs=1) as ph:
            hT = ph.tile([P, FC, NT], F32R)
            with tc.tile_pool(name="w1p", bufs=2) as w1_pool:
                for fg in range(DFF // 512):
                    w1t = w1_pool.tile([P, DM, 512], F32R)
                    nc.sync.dma_start(
                        w1t[:], _rearr(w1.ap()[:, fg * 512:(fg + 1) * 512]))
                    for f4 in range(4):
                        f = fg * 4 + f4
                        acc = psacc.tile([P, NT], F32, tag="acc")
                        for k in range(DM):
                            nc.tensor.matmul(
                                acc[:], w1t[:, k, f4 * P:(f4 + 1) * P],
                                xln1[:, k, :],
                                start=(k == 0), stop=(k == DM - 1))
                        nc.scalar.activation(hT[:, f, :], acc[:], AF.Gelu,
                                             bias=b1_sb[:, f:f + 1])

            with tc.tile_pool(name="w2p", bufs=2) as w2_pool:
                for ef in range(DM):
                    w2t = w2_pool.tile([P, FC, P], F32R)
                    nc.sync.dma_start(
                        w2t[:], w2.ap()[:, ef * P:(ef + 1) * P]
                        .rearrange("(c p) e -> p c e", p=P))
                    acc = psacc.tile([P, NT], F32, tag="acc")
                    for k in range(FC):
                        nc.tensor.matmul(acc[:], w2t[:, k, :], hT[:, k, :],
                                         start=(k == 0), stop=(k == FC - 1))
                    t = sq_pool.tile([P, NT], F32, tag="sq")
                    nc.vector.tensor_scalar_add(t[:], acc[:],
                                                b2_sb[:, ef:ef + 1])
                    nc.vector.tensor_add(z2[:, ef, :], t[:], xln1[:, ef, :])

        # -------- LN2 -> output ------------------------------------------
        with tc.tile_pool(name="outstage", bufs=2) as out_pool:
            yT_r = _rearr(yT_d.ap())

            def write_out(k, t):
                o = out_pool.tile([P, NT], F32)
                nc.scalar.activation(o[:], t[:], AF.Identity,
                                     scale=lnw2_sb[:, k:k + 1],
                                     bias=lnb2_sb[:, k:k + 1])
                nc.sync.dma_start(yT_r[:, k, :], o[:])
            ln_apply(z2, write_out)


def _get_nc():
    global _NC_CACHE
    if _NC_CACHE is None:
        _NC_CACHE = _build_nc()
    return _NC_CACHE


def kernel(x, w_qkv, w_out, ln1_w, ln1_b, w1, b1, w2, b2, ln2_w, ln2_b):
    x = np.ascontiguousarray(np.asarray(x, dtype=np.float32))
    shared = {
        "w_qkv": np.ascontiguousarray(np.asarray(w_qkv, np.float32)),
        "w_out": np.ascontiguousarray(np.asarray(w_out, np.float32)),
        "w1": np.ascontiguousarray(np.asarray(w1, np.float32)),
        "w2": np.ascontiguousarray(np.asarray(w2, np.float32)),
        "b1": np.asarray(b1, np.float32),
        "b2": np.asarray(b2, np.float32),
        "ln1_w": np.asarray(ln1_w, np.float32),
        "ln1_b": np.asarray(ln1_b, np.float32),
        "ln2_w": np.asarray(ln2_w, np.float32),
        "ln2_b": np.asarray(ln2_b, np.float32),
    }
    in_maps = []
    for c in range(8):
        b, q = divmod(c, 4)
        xT = np.ascontiguousarray(x[b].T)             # [D, NSEQ]
        # rotate so this core's own tokens are always columns [0, NT)
        xT = np.ascontiguousarray(np.roll(xT, -q * NT, axis=1))
        in_maps.append({"xT": xT, **shared})

    nc = _get_nc()
    res = run_bass_kernel_spmd(nc, in_maps, list(range(8)))

    out = np.empty((B, NSEQ, D), np.float32)
    for c in range(8):
        b, q = divmod(c, 4)
        out[b, q * NT:(q + 1) * NT, :] = res.results[c]["yT"].T
    return out
